# revision 1
# baseline (speedup 1.0000x reference)
"""Trainium2 Bass kernel for CrossAttention (B=8, N=M=2048, C=1024), fp32.

Sharding: data-parallel — one batch element per NeuronCore (8 cores).

Per-core computation (batch b):
  q  = x[b] @ wq^T          -> held transposed:  qT[d, n]
  kT[d, m] = (support[b] @ wk^T)^T
  v[m, d]  = (support[b] @ wv^T) * mask[m]   (post-softmax column mask == row
                                              mask on v; softmax denominator
                                              still spans all m)
  p  = exp(SCALE * qT^T kT)  (no max subtraction: logits ~ N(0, 8), safe fp32)
  o[n, d] = (p @ v) / rowsum(p)
  The reference's  out.swapaxes(1,2).reshape(N, C)  permutation satisfies
  o_perm[2t+i, c] = o[1024*i + c, t], so the final projection becomes
  final[2t+i, d'] = sum_c o[1024*i + c, t] * proj_w[d', c]  — a plain matmul
  with o-block-i rows as the contraction dim, written out with row stride 2.

Matmul operands are float32r (full-rate PE streaming; plain fp32 is 4 cyc/row).
Host-side prep transposes x/support/weights once (fp32 has no DMA-transpose on
TRN2) and lays weights out in consumption order so chunked DMAs pipeline with
the first accumulation groups at phase boundaries.
"""

import sys

sys.path.insert(0, "/opt/trn_rl_repo")

import numpy as np

import concourse.bass as bass
import concourse.tile as tile
from concourse import bacc, mybir
from concourse.bass_utils import run_bass_kernel_spmd
from concourse.masks import make_identity

F32 = mybir.dt.float32
F32R = mybir.dt.float32r
AF = mybir.ActivationFunctionType

B, N, M, C = 8, 2048, 2048, 1024
CT = C // 128          # 8 c-tiles (contraction / channel partition tiles)
MT = M // 128          # 16 m-tiles
SCALE = (C // 8) ** -0.5
NCHUNK = 256           # q rows computed per chunk in the attention phase
MS = 512               # m-chunk for kv build and the s matmul free dim
WCH = 16               # weight DMA chunks (arrival order == consumption order)

_CACHE = {}


def _build_program():
    nc = bacc.Bacc(
        "TRN2",
        target_bir_lowering=False,
        debug=False,
        enable_asserts=False,
        num_devices=8,
    )

    xT = nc.dram_tensor("xT", [128, CT, N], F32, kind="ExternalInput")
    sT = nc.dram_tensor("sT", [128, CT, M], F32, kind="ExternalInput")
    wqT = nc.dram_tensor("wqT", [128, CT * C], F32, kind="ExternalInput")
    wkT = nc.dram_tensor("wkT", [128, CT * C], F32, kind="ExternalInput")
    wvT = nc.dram_tensor("wvT", [128, CT * C], F32, kind="ExternalInput")
    pwT = nc.dram_tensor("pwT", [128, CT * C], F32, kind="ExternalInput")
    maskf = nc.dram_tensor("maskf", [128, MT], F32, kind="ExternalInput")
    biasb = nc.dram_tensor("biasb", [128, C], F32, kind="ExternalInput")
    out = nc.dram_tensor("out", [N, C], F32, kind="ExternalOutput")

    with tile.TileContext(nc, pool_alloc_mode="queue") as tc:
        _trace_kernel(tc, xT, sT, wqT, wkT, wvT, pwT, maskf, biasb, out)
    nc.compile()
    return nc


def _dma_w(nc, wtile, wdram, ch0=0, ch1=WCH):
    # chunked weight load: pipelines with the first consuming matmul groups
    cw = (CT * C) // WCH
    for ch in range(ch0, ch1):
        nc.sync.dma_start(
            wtile[:, ch * cw:(ch + 1) * cw],
            wdram[:, ch * cw:(ch + 1) * cw].bitcast(F32R),
        )


def _dma_act(nc, atile, adram, col0, cols):
    # per-ct chunked activation load (arrival order == psum-group order)
    for ct in range(CT):
        nc.sync.dma_start(
            atile[:, ct, :],
            adram[:, ct, col0:col0 + cols].bitcast(F32R),
        )


def _trace_kernel(tc, xT, sT, wqT, wkT, wvT, pwT, maskf, biasb, out):
    nc = tc.nc

    from contextlib import ExitStack

    with ExitStack() as ctx:
        persist = ctx.enter_context(tc.tile_pool(name="persist", bufs=1))

        ident0 = persist.tile([128, 128], F32, tag="ident0")
        make_identity(nc, ident0[:])
        ident = persist.tile([128, 128], F32R, tag="ident")
        nc.scalar.copy(ident[:], ident0[:])
        maskt = persist.tile([128, MT], F32, tag="maskt")
        nc.sync.dma_start(maskt[:], maskf[:])

        # o bounce buffer in DRAM (dependency-tracked tile)
        dram = ctx.enter_context(tc.tile_pool(name="dram", bufs=1, space="DRAM"))
        o_dram = dram.tile([N, C], F32, tag="o_dram")

        # v/kT live through attention; released before proj.  kT's pool opens
        # at phase K so phase V has room to preload wk alongside wv.
        kv_ctx = ctx.enter_context(ExitStack())
        vp = kv_ctx.enter_context(tc.tile_pool(name="vp", bufs=1))
        # v [m,d] as 16 col-blocks of [128, C]
        v = vp.tile([128, MT * C], F32R, tag="v")
        wk_ctx = ctx.enter_context(ExitStack())
        wkp = wk_ctx.enter_context(tc.tile_pool(name="wkp", bufs=1, side="right"))
        wk = wkp.tile([128, CT * C], F32R, tag="wk")

        # ---------------- phase V: v[m, d] = mask * (support @ wv^T) -------
        # wv is dc-major: [p, dc*4096 + ct*512 + dd]
        with (
            tc.tile_pool(name="wvp", bufs=1) as wvp,
            tc.tile_pool(name="stv", bufs=2) as stp,
            tc.tile_pool(name="vps", bufs=3, space="PSUM") as vps,
        ):
            wv = wvp.tile([128, CT * C], F32R, tag="wv")
            st0 = stp.tile([128, CT, MS], F32R, tag="st")
            cw = (CT * C) // WCH
            for i in range(CT):
                nc.sync.dma_start(
                    wv[:, i * cw:(i + 1) * cw],
                    wvT[:, i * cw:(i + 1) * cw].bitcast(F32R),
                )
                nc.sync.dma_start(
                    st0[:, i, :], sT[:, i, 0:MS].bitcast(F32R)
                )
            _dma_w(nc, wv, wvT, CT, WCH)
            for mc in range(M // MS):
                if mc == 0:
                    st = st0
                else:
                    st = stp.tile([128, CT, MS], F32R, tag="st")
                    _dma_act(nc, st, sT, mc * MS, MS)
                # spread next phase's weight prefetch across V's mc loop
                _dma_w(nc, wk, wkT, mc * 4, (mc + 1) * 4)
                for dc in range(C // 512):
                    for j in range(MS // 128):
                        mt = mc * (MS // 128) + j
                        ps = vps.tile([128, 512], F32, tag="vps")
                        for ct in range(CT):
                            nc.tensor.matmul(
                                ps[:],
                                lhsT=st[:, ct, j * 128:(j + 1) * 128],
                                rhs=wv[:, dc * 4096 + ct * 512: dc * 4096 + (ct + 1) * 512],
                                start=(ct == 0),
                                stop=(ct == CT - 1),
                            )
                        nc.vector.tensor_scalar_mul(
                            v[:, mt * C + dc * 512: mt * C + (dc + 1) * 512],
                            ps[:],
                            maskt[:, mt:mt + 1],
                        )

        # ---------------- phase K: kT[d, m] = (support @ wk^T)^T -----------
        # wk is dt-major: [p, dt*1024 + ct*128 + dd]; preloaded during V
        kTp = kv_ctx.enter_context(tc.tile_pool(name="kTp", bufs=1))
        # kT [d,m] as 8 col-blocks of [128, M]
        kT = kTp.tile([128, CT * M], F32R, tag="kT")
        with (
            tc.tile_pool(name="stk", bufs=2) as stp,
            tc.tile_pool(name="kps", bufs=3, space="PSUM") as kps,
        ):
            for mc in range(M // MS):
                st = stp.tile([128, CT, MS], F32R, tag="st")
                _dma_act(nc, st, sT, mc * MS, MS)
                for dt in range(CT):
                    ps = kps.tile([128, MS], F32, tag="kps")
                    for ct in range(CT):
                        nc.tensor.matmul(
                            ps[:],
                            lhsT=wk[:, dt * C + ct * 128: dt * C + (ct + 1) * 128],
                            rhs=st[:, ct, :],
                            start=(ct == 0),
                            stop=(ct == CT - 1),
                        )
                    nc.scalar.copy(
                        kT[:, dt * M + mc * MS: dt * M + (mc + 1) * MS], ps[:]
                    )

        wk_ctx.close()

        # ---------------- attention: per n-chunk qT, then s/p/o ------------
        # wq is dt-major like wk
        with (
            tc.tile_pool(name="wqp", bufs=1) as wqp,
            tc.tile_pool(name="xq", bufs=1) as xqp,
            tc.tile_pool(name="qt", bufs=1) as qtp,
            tc.tile_pool(name="qps", bufs=2, space="PSUM") as qps,
            tc.tile_pool(name="sps", bufs=2, space="PSUM") as sps,
            tc.tile_pool(name="ptps", bufs=2, space="PSUM") as ptps,
            tc.tile_pool(name="ops", bufs=1, space="PSUM") as ops,
            tc.tile_pool(name="psb", bufs=2) as psbp,
            tc.tile_pool(name="ptsb", bufs=2) as ptsbp,
            tc.tile_pool(name="osb", bufs=2) as osbp,
            tc.tile_pool(name="stat", bufs=4) as statp,
        ):
            wq = wqp.tile([128, CT * C], F32R, tag="wq")
            _dma_w(nc, wq, wqT, 0, 2)  # dt0 block: first qT group's weights
            xq0 = xqp.tile([128, CT, NCHUNK], F32R, tag="xq")
            _dma_act(nc, xq0, xT, 0, NCHUNK)
            _dma_w(nc, wq, wqT, 2, WCH)
            for nch in range(N // NCHUNK):
                if nch == 0:
                    xq = xq0
                else:
                    xq = xqp.tile([128, CT, NCHUNK], F32R, tag="xq")
                    _dma_act(nc, xq, xT, nch * NCHUNK, NCHUNK)
                qt = qtp.tile([128, CT * NCHUNK], F32R, tag="qt")
                for dt in range(CT):
                    ps = qps.tile([128, NCHUNK], F32, tag="qps")
                    for ct in range(CT):
                        nc.tensor.matmul(
                            ps[:],
                            lhsT=wq[:, dt * C + ct * 128: dt * C + (ct + 1) * 128],
                            rhs=xq[:, ct, :],
                            start=(ct == 0),
                            stop=(ct == CT - 1),
                        )
                    nc.scalar.copy(
                        qt[:, dt * NCHUNK:(dt + 1) * NCHUNK], ps[:]
                    )
                for nt2 in range(NCHUNK // 128):
                    ntile = nch * (NCHUNK // 128) + nt2
                    partials = statp.tile([128, 4], F32, tag="partials")
                    o_ps = ops.tile([128, C], F32, tag="ops")
                    for g in range(M // MS):
                        s_ps = sps.tile([128, MS], F32, tag="sps")
                        for dt in range(CT):
                            nc.tensor.matmul(
                                s_ps[:],
                                lhsT=qt[:, dt * NCHUNK + nt2 * 128: dt * NCHUNK + (nt2 + 1) * 128],
                                rhs=kT[:, dt * M + g * MS: dt * M + (g + 1) * MS],
                                start=(dt == 0),
                                stop=(dt == CT - 1),
                            )
                        p_sb = psbp.tile([128, MS], F32R, tag="psb")
                        nc.scalar.activation(
                            p_sb[:], s_ps[:], AF.Exp,
                            scale=float(SCALE),
                            accum_out=partials[:, g:g + 1],
                        )
                        pt_ps = ptps.tile([128, MS], F32R, tag="ptps")
                        for j in range(MS // 128):
                            nc.tensor.transpose(
                                pt_ps[:, j * 128:(j + 1) * 128],
                                p_sb[:, j * 128:(j + 1) * 128],
                                ident[:],
                            )
                        pt_sb = ptsbp.tile([128, MS], F32R, tag="ptsb")
                        nc.vector.tensor_copy(pt_sb[:], pt_ps[:])
                        for j in range(MS // 128):
                            mt = g * (MS // 128) + j
                            for dc in range(C // 512):
                                nc.tensor.matmul(
                                    o_ps[:, dc * 512:(dc + 1) * 512],
                                    lhsT=pt_sb[:, j * 128:(j + 1) * 128],
                                    rhs=v[:, mt * C + dc * 512: mt * C + (dc + 1) * 512],
                                    start=(mt == 0),
                                    stop=(mt == MT - 1),
                                )
                    denom = statp.tile([128, 1], F32, tag="denom")
                    nc.vector.reduce_sum(
                        denom[:], partials[:], axis=mybir.AxisListType.X
                    )
                    recip = statp.tile([128, 1], F32, tag="recip")
                    nc.vector.reciprocal(recip[:], denom[:])
                    o_sb = osbp.tile([128, C], F32, tag="osb")
                    nc.vector.tensor_scalar_mul(o_sb[:], o_ps[:], recip[:])
                    nc.sync.dma_start(
                        o_dram[ntile * 128:(ntile + 1) * 128, :], o_sb[:]
                    )

        kv_ctx.close()

        # ---------------- projection with the swapaxes/reshape fold --------
        # pw is dc-major like wv
        with (
            tc.tile_pool(name="pwp", bufs=1) as pwp,
            tc.tile_pool(name="bp", bufs=1) as bp,
            tc.tile_pool(name="obp", bufs=2) as obp,
            tc.tile_pool(name="fps", bufs=2, space="PSUM") as fps,
            tc.tile_pool(name="fsb", bufs=2) as fsbp,
        ):
            pw = pwp.tile([128, CT * C], F32R, tag="pw")
            bias = bp.tile([128, C], F32, tag="bias")
            ob0 = obp.tile([128, CT * C], F32R, tag="ob")
            cw = (CT * C) // WCH
            for i in range(CT):
                nc.sync.dma_start(
                    pw[:, i * cw:(i + 1) * cw],
                    pwT[:, i * cw:(i + 1) * cw].bitcast(F32R),
                )
                # plain 2D slices: a rearranged AP on a DRAM pool tile defeats
                # Tile's RAW dep tracking (read would race the o_dram writes)
                nc.sync.dma_start(
                    ob0[:, i * C:(i + 1) * C],
                    o_dram[i * 128:(i + 1) * 128, :].bitcast(F32R),
                )
            _dma_w(nc, pw, pwT, CT, WCH)
            nc.sync.dma_start(bias[:], biasb[:])
            out_v = out[:].rearrange("(t two) d -> two t d", two=2)
            for i in range(2):
                if i == 0:
                    ob = ob0
                else:
                    ob = obp.tile([128, CT * C], F32R, tag="ob")
                    for ct in range(CT):
                        nc.sync.dma_start(
                            ob[:, ct * C:(ct + 1) * C],
                            o_dram[i * C + ct * 128: i * C + (ct + 1) * 128, :].bitcast(F32R),
                        )
                for dc in range(C // 512):
                    for tt in range(CT):
                        ps = fps.tile([128, 512], F32, tag="fps")
                        for ct in range(CT):
                            nc.tensor.matmul(
                                ps[:],
                                lhsT=ob[:, ct * C + tt * 128: ct * C + (tt + 1) * 128],
                                rhs=pw[:, dc * 4096 + ct * 512: dc * 4096 + (ct + 1) * 512],
                                start=(ct == 0),
                                stop=(ct == CT - 1),
                            )
                        f_sb = fsbp.tile([128, 512], F32, tag="fsb")
                        nc.vector.tensor_add(
                            f_sb[:], ps[:], bias[:, dc * 512:(dc + 1) * 512]
                        )
                        nc.sync.dma_start(
                            out_v[i, tt * 128:(tt + 1) * 128, dc * 512:(dc + 1) * 512],
                            f_sb[:],
                        )


def _prep_w_lhs(w):
    # lhsT weights (wk, wq): dt-major [p, dt*1024 + ct*128 + dd]
    wt = w.T.reshape(CT, 128, CT, 128)          # [ct, p, dt, dd]
    return np.ascontiguousarray(
        wt.transpose(1, 2, 0, 3).reshape(128, CT * C)
    )


def _prep_w_rhs(w):
    # rhs weights (wv, pw): dc-major [p, dc*4096 + ct*512 + dd]
    wt = w.T.reshape(CT, 128, C // 512, 512)    # [ct, p, dc, dd]
    return np.ascontiguousarray(
        wt.transpose(1, 2, 0, 3).reshape(128, CT * C)
    )


def _prep_act(a):
    # a [rows, C] -> a.T [C, rows] grouped as [p, ct, rows]
    n = a.shape[0]
    return np.ascontiguousarray(a.T.reshape(CT, 128, n).transpose(1, 0, 2))


def prep_in_maps(x, support, attn_mask, qkv_w, proj_w, proj_b):
    x = np.asarray(x, dtype=np.float32)
    support = np.asarray(support, dtype=np.float32)
    attn_mask = np.asarray(attn_mask)
    qkv_w = np.asarray(qkv_w, dtype=np.float32)
    proj_w = np.asarray(proj_w, dtype=np.float32)
    proj_b = np.asarray(proj_b, dtype=np.float32)

    wq = _prep_w_lhs(qkv_w[:C])
    wk = _prep_w_lhs(qkv_w[C:2 * C])
    wv = _prep_w_rhs(qkv_w[2 * C:])
    pw = _prep_w_rhs(proj_w)
    maskf = np.ascontiguousarray(
        attn_mask.astype(np.float32).reshape(MT, 128).T
    )
    biasb = np.ascontiguousarray(np.broadcast_to(proj_b, (128, C)))

    in_maps = []
    for b in range(B):
        in_maps.append({
            "xT": _prep_act(x[b]),
            "sT": _prep_act(support[b]),
            "wqT": wq,
            "wkT": wk,
            "wvT": wv,
            "pwT": pw,
            "maskf": maskf,
            "biasb": biasb,
        })
    return in_maps


def kernel(x, support, attn_mask, qkv_w, proj_w, proj_b):
    if "nc" not in _CACHE:
        _CACHE["nc"] = _build_program()
    nc = _CACHE["nc"]

    in_maps = prep_in_maps(x, support, attn_mask, qkv_w, proj_w, proj_b)
    res = run_bass_kernel_spmd(nc, in_maps, core_ids=list(range(B)))
    return np.stack([res.results[b]["out"] for b in range(B)], axis=0)



# revision 9
# speedup vs baseline: 1.6553x; 1.6553x over previous
"""Trainium2 Bass kernel for CrossAttention (B=8, N=M=2048, C=1024), fp32 io.

Sharding: data-parallel -- one batch element per NeuronCore (8 cores).

Per-core math (batch b), bf16 matmul datapath with fp32 PSUM accumulation
(same PE rate as float32r, half the DMA/SBUF, and no >=256-free-dim
constraint on full-rate matmuls):

  s = q k^T = x Wq^T Wk support^T = x W' support^T   with W' = Wq^T Wk
  computed in fp32 on HOST -- so the q projection GEMM disappears and the
  kernel builds only  kT2[j, m] = (support @ W'^T)^T,  consuming the
  already-transposed input xT[j, n] directly as the s-matmul rhs:
    sT[m, n] = kT2^T xT    (per m-tile; exp(SCALE*sT) -> pT is directly
                            the lhsT of the p@v matmul: no PE transposes)
  v[m, d]  = mask[m] * (support[b] @ wv^T)
  o[n, d]  = (p @ v) / denom[n],  denom[n] = sum_m exp(SCALE*s[n, m])

Structural tricks:
  * The m axis is pre-sorted on host so all attn_mask==1 rows come first.
    Softmax's denominator is permutation-invariant and the reference masks
    AFTER softmax, so only A = ceil(ones/128) m-tiles contribute to v and
    p@v; fully-masked tiles are skipped there (but still feed the
    denominator via exp(sT)).
  * denom is accumulated with [128,1] ones-matmuls over all 16 pT tiles
    (ap_size=1 -> ~free on the PE).
  * o stays resident in SBUF. The reference's swapaxes(1,2).reshape fold
    means proj consumes o's [token-part, feature-free] tiles directly as
    lhsT (contraction over token blocks), so there is no DRAM bounce and
    no transpose in the projection either:
      final[2t+i, d'] = sum_c o[1024*i + c, t] * proj_w[d', c]
    (proj contracts over o's TOKEN axis, which is also why Wv/proj_w can't
    be host-folded like Wq/Wk.)
"""

import sys

sys.path.insert(0, "/opt/trn_rl_repo")

import ml_dtypes
import numpy as np

import concourse.bass as bass
import concourse.tile as tile
from concourse import bacc, mybir
from concourse.bass_utils import run_bass_kernel_spmd

F32 = mybir.dt.float32
BF16 = mybir.dt.bfloat16
AF = mybir.ActivationFunctionType
NPBF = ml_dtypes.bfloat16

B, N, M, C = 8, 2048, 2048, 1024
CT = C // 128           # 8 contraction/partition tiles of the channel dim
MT = M // 128           # 16 m-tiles
NT = N // 128           # 16 n-tiles
NCH = 512               # n columns per attention chunk
SCALE = (C // 8) ** -0.5

_CACHE = {}


def _build_program(A):
    nc = bacc.Bacc(
        "TRN2",
        target_bir_lowering=False,
        debug=False,
        enable_asserts=False,
        num_devices=8,
    )

    xT = nc.dram_tensor("xT", [128, CT, N], BF16, kind="ExternalInput")
    sT = nc.dram_tensor("sT", [128, CT, M], BF16, kind="ExternalInput")
    w2T = nc.dram_tensor("w2T", [128, CT * C], BF16, kind="ExternalInput")
    wvT = nc.dram_tensor("wvT", [128, CT * C], BF16, kind="ExternalInput")
    pwT = nc.dram_tensor("pwT", [128, CT * C], BF16, kind="ExternalInput")
    maskf = nc.dram_tensor("maskf", [128, MT], F32, kind="ExternalInput")
    biasb = nc.dram_tensor("biasb", [128, C], F32, kind="ExternalInput")
    out = nc.dram_tensor("out", [N, C], F32, kind="ExternalOutput")

    with tile.TileContext(nc, pool_alloc_mode="queue") as tc:
        _trace_kernel(tc, A, xT, sT, w2T, wvT, pwT, maskf, biasb, out)
    nc.compile()
    return nc


def _trace_kernel(tc, A, xT, sT, w2T, wvT, pwT, maskf, biasb, out):
    nc = tc.nc

    from contextlib import ExitStack

    with ExitStack() as ctx:
        persist = ctx.enter_context(tc.tile_pool(name="persist", bufs=1))
        maskt = persist.tile([128, MT], F32, tag="maskt")
        ones = persist.tile([128, 1], BF16, tag="ones")
        nc.vector.memset(ones[:], 1.0)

        # persistent activations (live across phases)
        qkp = ctx.enter_context(tc.tile_pool(name="qkp", bufs=1))
        kT2 = qkp.tile([128, CT * M], BF16, tag="kT2")
        xsb = qkp.tile([128, CT, N], BF16, tag="xsb")

        # support^T stays resident across phases K and V
        ss_ctx = ctx.enter_context(ExitStack())
        ssp = ss_ctx.enter_context(tc.tile_pool(name="ssp", bufs=1, side="right"))
        ssb = ssp.tile([128, CT, M], BF16, tag="ssb")
        wp = ss_ctx.enter_context(tc.tile_pool(name="wp", bufs=1, side="right"))
        w2 = wp.tile([128, CT * C], BF16, tag="w2")
        wv = wp.tile([128, CT * C], BF16, tag="wv")

        # DMA issue order == consumption order (transfers serialize in the
        # DMA-engine pool, so put near-term data first)
        nc.sync.dma_start(w2[:, 0:C], w2T[:, 0:C])
        nc.sync.dma_start(ssb[:, :, 0:NCH], sT[:, :, 0:NCH])
        nc.sync.dma_start(w2[:, C:CT * C], w2T[:, C:CT * C])
        for mc in range(1, M // NCH):
            nc.sync.dma_start(
                ssb[:, :, mc * NCH:(mc + 1) * NCH], sT[:, :, mc * NCH:(mc + 1) * NCH]
            )
        nc.sync.dma_start(wv[:], wvT[:])
        for nch in range(N // NCH):
            nc.sync.dma_start(
                xsb[:, :, nch * NCH:(nch + 1) * NCH], xT[:, :, nch * NCH:(nch + 1) * NCH]
            )
        nc.sync.dma_start(maskt[:], maskf[:])

        # ---------------- phase K: kT2[j, m] = (support @ W'^T)^T -----------
        # W' is dt-major: [p, dt*1024 + ct*128 + dd]
        with tc.tile_pool(name="kps", bufs=3, space="PSUM") as kps:
            for mc in range(M // NCH):
                for dt in range(CT):
                    ps = kps.tile([128, NCH], F32, tag="kps")
                    for ct in range(CT):
                        nc.tensor.matmul(
                            ps[:],
                            lhsT=w2[:, dt * C + ct * 128: dt * C + (ct + 1) * 128],
                            rhs=ssb[:, ct, mc * NCH:(mc + 1) * NCH],
                            start=(ct == 0),
                            stop=(ct == CT - 1),
                        )
                    nc.scalar.copy(
                        kT2[:, dt * M + mc * NCH: dt * M + (mc + 1) * NCH], ps[:]
                    )

        # ---------------- phase V: v[m, d] = mask * (support @ wv^T) --------
        # only the A active (mask-sorted) m-tiles; wv is dc-major
        vp = ctx.enter_context(tc.tile_pool(name="vp", bufs=1))
        v = vp.tile([128, A * C], BF16, tag="v")
        with tc.tile_pool(name="vps", bufs=3, space="PSUM") as vps:
            for mt in range(A):
                for dc in range(C // 512):
                    ps = vps.tile([128, 512], F32, tag="vps")
                    for ct in range(CT):
                        nc.tensor.matmul(
                            ps[:],
                            lhsT=ssb[:, ct, mt * 128:(mt + 1) * 128],
                            rhs=wv[:, dc * 4096 + ct * 512: dc * 4096 + (ct + 1) * 512],
                            start=(ct == 0),
                            stop=(ct == CT - 1),
                        )
                    nc.vector.tensor_scalar_mul(
                        v[:, mt * C + dc * 512: mt * C + (dc + 1) * 512],
                        ps[:],
                        maskt[:, mt:mt + 1],
                    )

        ss_ctx.close()

        # ---------------- attention: sT/exp per m-tile, then p@v ------------
        pw_ctx = ctx.enter_context(ExitStack())
        pwp = pw_ctx.enter_context(tc.tile_pool(name="pwp", bufs=1, side="right"))
        pw = pwp.tile([128, CT * C], BF16, tag="pw")
        bias = pwp.tile([128, C], F32, tag="bias")
        nc.sync.dma_start(pw[:], pwT[:])
        nc.sync.dma_start(bias[:], biasb[:])

        op = ctx.enter_context(tc.tile_pool(name="op", bufs=1))
        o_sb = op.tile([128, NT * C], BF16, tag="o_sb")

        with (
            tc.tile_pool(name="ptp", bufs=2) as ptp,
            tc.tile_pool(name="sps", bufs=2, space="PSUM") as sps,
            tc.tile_pool(name="ops", bufs=2, space="PSUM") as ops,
            tc.tile_pool(name="dps", bufs=2, space="PSUM") as dps,
            tc.tile_pool(name="stat", bufs=4) as statp,
        ):
            for nch in range(N // NCH):
                pT = ptp.tile([128, MT * NCH], BF16, tag="pT")
                for mt in range(MT):
                    s_ps = sps.tile([128, NCH], F32, tag="sps")
                    for ct in range(CT):
                        nc.tensor.matmul(
                            s_ps[:],
                            lhsT=kT2[:, ct * M + mt * 128: ct * M + (mt + 1) * 128],
                            rhs=xsb[:, ct, nch * NCH:(nch + 1) * NCH],
                            start=(ct == 0),
                            stop=(ct == CT - 1),
                        )
                    nc.scalar.activation(
                        pT[:, mt * NCH:(mt + 1) * NCH], s_ps[:], AF.Exp,
                        scale=float(SCALE),
                    )
                for j in range(NCH // 128):
                    ntile = nch * (NCH // 128) + j
                    o_ps = ops.tile([128, C], F32, tag="ops")
                    for mt in range(A):
                        for dc in range(C // 512):
                            nc.tensor.matmul(
                                o_ps[:, dc * 512:(dc + 1) * 512],
                                lhsT=pT[:, mt * NCH + j * 128: mt * NCH + (j + 1) * 128],
                                rhs=v[:, mt * C + dc * 512: mt * C + (dc + 1) * 512],
                                start=(mt == 0),
                                stop=(mt == A - 1),
                            )
                    # denominator spans ALL m-tiles (masked ones included)
                    d_ps = dps.tile([128, 1], F32, tag="dps")
                    for mt in range(MT):
                        nc.tensor.matmul(
                            d_ps[:],
                            lhsT=pT[:, mt * NCH + j * 128: mt * NCH + (j + 1) * 128],
                            rhs=ones[:],
                            start=(mt == 0),
                            stop=(mt == MT - 1),
                        )
                    recip = statp.tile([128, 1], F32, tag="recip")
                    nc.vector.reciprocal(recip[:], d_ps[:])
                    nc.vector.tensor_scalar_mul(
                        o_sb[:, ntile * C:(ntile + 1) * C], o_ps[:], recip[:]
                    )

        # ---------------- projection with the swapaxes/reshape fold ---------
        # final[2t+i, d'] = sum_c o[1024*i + c, t] * proj_w[d', c]:
        # lhsT = o tiles (token partitions, feature free), rhs = pw dc-major
        with (
            tc.tile_pool(name="fps", bufs=4, space="PSUM") as fps,
            tc.tile_pool(name="fsb", bufs=3) as fsbp,
        ):
            out_v = out[:].rearrange("(t two) d -> two t d", two=2)
            for i in range(2):
                for tt in range(CT):
                    for dc in range(C // 512):
                        ps = fps.tile([128, 512], F32, tag="fps")
                        for ct in range(CT):
                            nc.tensor.matmul(
                                ps[:],
                                lhsT=o_sb[:, (i * CT + ct) * C + tt * 128:
                                          (i * CT + ct) * C + (tt + 1) * 128],
                                rhs=pw[:, dc * 4096 + ct * 512: dc * 4096 + (ct + 1) * 512],
                                start=(ct == 0),
                                stop=(ct == CT - 1),
                            )
                        f_sb = fsbp.tile([128, 512], F32, tag="fsb")
                        nc.vector.tensor_add(
                            f_sb[:], ps[:], bias[:, dc * 512:(dc + 1) * 512]
                        )
                        nc.sync.dma_start(
                            out_v[i, tt * 128:(tt + 1) * 128, dc * 512:(dc + 1) * 512],
                            f_sb[:],
                        )

        pw_ctx.close()


def _prep_w_lhs(w):
    # lhsT weights (W'): dt-major [p, dt*1024 + ct*128 + dd]
    wt = w.T.reshape(CT, 128, CT, 128)          # [ct, p, dt, dd]
    return np.ascontiguousarray(
        wt.transpose(1, 2, 0, 3).reshape(128, CT * C).astype(NPBF)
    )


def _prep_w_rhs(w):
    # rhs weights (wv, pw): dc-major [p, dc*4096 + ct*512 + dd]
    wt = w.T.reshape(CT, 128, C // 512, 512)    # [ct, p, dc, dd]
    return np.ascontiguousarray(
        wt.transpose(1, 2, 0, 3).reshape(128, CT * C).astype(NPBF)
    )


def _prep_act(a):
    # a [rows, C] -> a.T [C, rows] grouped as [p, ct, rows]
    n = a.shape[0]
    return np.ascontiguousarray(
        a.T.reshape(CT, 128, n).transpose(1, 0, 2).astype(NPBF)
    )


def prep_in_maps(x, support, attn_mask, qkv_w, proj_w, proj_b):
    x = np.asarray(x, dtype=np.float32)
    support = np.asarray(support, dtype=np.float32)
    attn_mask = np.asarray(attn_mask)
    qkv_w = np.asarray(qkv_w, dtype=np.float32)
    proj_w = np.asarray(proj_w, dtype=np.float32)
    proj_b = np.asarray(proj_b, dtype=np.float32)

    # sort the m axis: unmasked rows first (softmax denom is order-invariant,
    # mask applies post-softmax, so masked m-tiles drop out of v and p@v)
    perm = np.argsort(-attn_mask, kind="stable")
    mask_s = np.ascontiguousarray(attn_mask[perm])
    support_s = support[:, perm, :]

    # host-fold the q/k projections: s = x (Wq^T Wk) support^T
    w2 = qkv_w[:C].T @ qkv_w[C:2 * C]           # [j (x chan), c' (supp chan)]
    w2p = _prep_w_lhs(w2)
    wv = _prep_w_rhs(qkv_w[2 * C:])
    pw = _prep_w_rhs(proj_w)
    maskf = np.ascontiguousarray(
        mask_s.astype(np.float32).reshape(MT, 128).T
    )
    biasb = np.ascontiguousarray(np.broadcast_to(proj_b, (128, C)))

    in_maps = []
    for b in range(B):
        in_maps.append({
            "xT": _prep_act(x[b]),
            "sT": _prep_act(support_s[b]),
            "w2T": w2p,
            "wvT": wv,
            "pwT": pw,
            "maskf": maskf,
            "biasb": biasb,
        })
    return in_maps


def _active_tiles(attn_mask):
    n_ones = int(np.asarray(attn_mask).sum())
    return max(1, (n_ones + 127) // 128)


def kernel(x, support, attn_mask, qkv_w, proj_w, proj_b):
    A = _active_tiles(attn_mask)
    if _CACHE.get("A") != A:
        _CACHE["nc"] = _build_program(A)
        _CACHE["A"] = A
    nc = _CACHE["nc"]

    in_maps = prep_in_maps(x, support, attn_mask, qkv_w, proj_w, proj_b)
    res = run_bass_kernel_spmd(nc, in_maps, core_ids=list(range(B)))
    return np.stack([res.results[b]["out"] for b in range(B)], axis=0)
